# revision 36
# baseline (speedup 1.0000x reference)
"""Causal multi-head self-attention with RoPE on 8 Trainium2 NeuronCores.

Sharding: grid of 4 batches x 2 head-groups (8 heads each). Core c handles
batch c//2, heads (c%2)*8..(c%2)*8+8. Each core computes its partial output
projection (S, D); the host sums the two head-group partials per batch.

Weights are passed pre-transposed; wq/wk additionally have a per-head
even/odd de-interleave column permutation so RoPE on device is expressible
with contiguous 32-wide blocks. RoPE cos/sin are gathered host-side from
token_positions.

Device schedule: attention for q-chunk qc is emitted interleaved with the
projection/rope/transpose work of q-chunk qc+1 and the output projection of
qc-1, so the PE never drains (keeps the tensor engine at its high p-state)
while the scalar engine runs the softmax exps. QK->exp->PV is software-
pipelined with a 2-tile lag. q/k transposes and attention operands are bf16
(full-rate PE); projections are float32r. Softmax skips max-subtraction
(logits are O(1) by construction); denominators come free from an appended
ones-column in V, inverted with the fast DVE reciprocal approximation.
"""
import math
from collections import deque

import numpy as np

import concourse.bass as bass
import concourse.mybir as mybir
import concourse.tile as tile
from concourse import bacc
from concourse.bass_utils import run_bass_kernel_spmd
from concourse.masks import make_identity

F32 = mybir.dt.float32
F32R = mybir.dt.float32r
BF16 = mybir.dt.bfloat16
EXP = mybir.ActivationFunctionType.Exp

D_MODEL = 1024
NUM_HEADS = 16
HEAD_DIM = 64
THETA = 10000.0
MAX_SEQ_LEN = 2048
BATCH = 4
S = 2048
H_LOC = 8           # heads per core
NI = 8              # contraction chunks of 128 over D_MODEL
ST = 16             # s-tiles of 128
QC = 4              # q-chunks of 512
N_CORES = 8

_PROGRAM_CACHE = {}
INTERLEAVE = True
DEBUG_DUMPS = False


def _build_program():
    nc = bacc.Bacc("TRN2", target_bir_lowering=False, debug=False,
                   num_devices=N_CORES)

    xT = nc.dram_tensor("xT", [D_MODEL, S], BF16, kind="ExternalInput")
    wqk = nc.dram_tensor("wqk", [D_MODEL, 1024], BF16, kind="ExternalInput")
    wv = nc.dram_tensor("wv", [D_MODEL, 512], BF16, kind="ExternalInput")
    woT = nc.dram_tensor("woT", [512, D_MODEL], BF16, kind="ExternalInput")
    cosD = nc.dram_tensor("cosD", [S, 64], F32, kind="ExternalInput")
    sinS = nc.dram_tensor("sinS", [S, 64], F32, kind="ExternalInput")
    out = nc.dram_tensor("out", [S, D_MODEL], F32, kind="ExternalOutput")
    if DEBUG_DUMPS:
        dbg_potS = nc.dram_tensor("dbg_potS", [16, 65, 1024], F32,
                                  kind="ExternalOutput")
        dbg_rr = nc.dram_tensor("dbg_rr", [16, 1024], F32, kind="ExternalOutput")
        dbg_kT = nc.dram_tensor("dbg_kT", [128, 4 * S], BF16,
                                kind="ExternalOutput")
        dbg_qT = nc.dram_tensor("dbg_qT", [2, 128, 2048], BF16,
                                kind="ExternalOutput")
        dbg_Vp = nc.dram_tensor("dbg_Vp", [4, 128, H_LOC * 65], BF16,
                                kind="ExternalOutput")

    with tile.TileContext(nc) as tc:
        with (
            tc.tile_pool(name="const", bufs=1) as cp,
            tc.tile_pool(name="stream", bufs=1) as sp,
            tc.tile_pool(name="psum", bufs=1, space="PSUM") as pp,
        ):
            # ---- resident constants -------------------------------------
            # DMA issue order is critical for startup latency: wqk chunks
            # first (gate the first q/k matmuls), then the first x chunk,
            # then wv; woT is deferred until attention(0) starts.
            # batched weight tiles: one dma_start each (a dma_start costs
            # ~0.7us of issue time on the sync sequencer, so few big DMAs
            # beat many small ones at startup)
            wqk_t = cp.tile([128, NI, 1024], BF16, tag="wqk", name="wqk_t")
            wv_t = cp.tile([128, NI, 512], BF16, tag="wv", name="wv_t")
            woT_t = cp.tile([128, 4, 1024], BF16, tag="woT", name="woT_t")
            # x resident in SBUF, one DMA per q-chunk: [128, ic, 512 s]
            xc = [cp.tile([128, NI, 512], BF16, tag="xc", bufs=QC, name=f"xc{i}")
                  for i in range(QC)]
            xT_r = xT[:].rearrange("(i p) s -> p i s", p=128)
            nc.sync.dma_start(wqk_t[:], wqk[:].rearrange("(i p) c -> p i c", p=128))
            nc.sync.dma_start(xc[0][:], xT_r[:, :, 0:512])
            nc.sync.dma_start(wv_t[:], wv[:].rearrange("(i p) c -> p i c", p=128))

            ident = cp.tile([128, 128], F32, tag="ident")
            make_identity(nc, ident[:])
            identB = cp.tile([128, 128], BF16, tag="identB")
            nc.vector.tensor_copy(identB[:], ident[:])
            tri01 = cp.tile([128, 128], F32, tag="tri01")
            nc.gpsimd.memset(tri01[:], 1.0)
            nc.gpsimd.affine_select(      # keep q >= t, zero q < t
                out=tri01[:], in_=tri01[:], compare_op=mybir.AluOpType.is_ge,
                fill=0.0, base=0, pattern=[[1, 128]], channel_multiplier=-1)
            tri01b = cp.tile([128, 128], BF16, tag="tri01b")
            nc.vector.tensor_copy(tri01b[:], tri01[:])

            # HAM warm-up: ~4us of dependency-free matmuls so the PE clock
            # gate opens during the initial weight/x DMA wait instead of
            # penalizing the first real s-tiles.
            warm = pp.tile([128, 512], F32, tag="sc", bufs=2, name="warm")
            for _ in range(90):
                nc.tensor.transpose(warm[:, 0:128], ident[:], ident[:])

            # kT: [128 dims(pair), 4 pair-blocks x 2048 t] bf16, resident
            kT_all = cp.tile([128, 4 * S], BF16, tag="kT")
            # qT triple-buffered (qc mod 3) so stage-A can run two chunks
            # ahead of attention: [128 dims(pair), 4 x 512 q]
            qT = [cp.tile([128, 4 * 512], BF16, tag="qT", bufs=3, name=f"qT{i}")
                  for i in range(3)]
            # SBUF accumulators for the last chunk's output projection
            po_sb = [cp.tile([128, 1024], F32, tag="posb", bufs=4,
                             name=f"posb{i}") for i in range(4)]
            # V' tiles: [128 t, 8 heads x (64 v + ones)]
            Vp = [cp.tile([128, H_LOC * 65], BF16, tag="Vp", bufs=ST, name=f"Vp{i}")
                  for i in range(ST)]
            for st in range(ST):
                nc.gpsimd.memset(
                    Vp[st][:].rearrange("p (h c) -> p h c", h=H_LOC)[:, :, 64:65],
                    1.0)

            OT_store = {}

            # ---- stage A: projections + rope + transposes ----------------
            def rope_apply(psrc, cos_t, sin_t, name):
                """psum [128 s, 512 (8h de-interleaved)] -> bf16 sbuf roped."""
                cos_b = (cos_t[:].rearrange("p (b i) -> p b i", b=2)
                         .unsqueeze(1).broadcast_to([128, 8, 2, 32]))
                sin_b = (sin_t[:].rearrange("p (b i) -> p b i", b=2)
                         .unsqueeze(1).broadcast_to([128, 8, 2, 32]))
                p4 = psrc[:].rearrange("p (h b i) -> p h b i", h=8, b=2)
                t1 = sp.tile([128, 512], F32, tag="ropet1", bufs=2, name=f"t1{name}")
                nc.vector.tensor_mul(
                    t1[:].rearrange("p (h b i) -> p h b i", h=8, b=2),
                    p4[:, :, ::-1, :], sin_b)
                t2 = sp.tile([128, 512], F32, tag="ropet2", bufs=2, name=f"t2{name}")
                nc.vector.tensor_mul(
                    t2[:].rearrange("p (h b i) -> p h b i", h=8, b=2),
                    p4, cos_b)
                r = sp.tile([128, 512], BF16, tag="roped", bufs=4, name=f"r{name}")
                nc.vector.tensor_add(r[:], t1[:], t2[:])
                return r

            def gen_stage_a(qc):
                qTd = qT[qc % 3]
                if qc > 0:
                    nc.sync.dma_start(xc[qc][:],
                                      xT_r[:, :, 512 * qc:512 * (qc + 1)])
                for stL in range(4):
                    st = 4 * qc + stL
                    xv = xc[qc][:, :, 128 * stL:128 * (stL + 1)]
                    # U1: q projection
                    cos_t = sp.tile([128, 64], F32, tag="cos", bufs=2, name=f"cos{st}")
                    sin_t = sp.tile([128, 64], F32, tag="sin", bufs=2, name=f"sin{st}")
                    nc.sync.dma_start(cos_t[:], cosD[128 * st:128 * (st + 1), :])
                    nc.sync.dma_start(sin_t[:], sinS[128 * st:128 * (st + 1), :])
                    pq = pp.tile([128, 512], F32, tag="stagea", bufs=2, name=f"pq{st}")
                    for ic in range(NI):
                        nc.tensor.matmul(pq[:], xv[:, ic, :], wqk_t[:, ic, 0:512],
                                         start=(ic == 0), stop=(ic == NI - 1))
                    yield
                    # U2: k-projection + rope(q)
                    pk = pp.tile([128, 512], F32, tag="stagea", bufs=2, name=f"pk{st}")
                    for ic in range(NI):
                        nc.tensor.matmul(pk[:], xv[:, ic, :], wqk_t[:, ic, 512:1024],
                                         start=(ic == 0), stop=(ic == NI - 1))
                    qr = rope_apply(pq, cos_t, sin_t, f"q{st}")
                    yield
                    # U3: rope(k), transposes of q and k, fused copies out
                    kr = rope_apply(pk, cos_t, sin_t, f"k{st}")
                    trq = pp.tile([128, 512], BF16, tag="stagea", bufs=2,
                                  name=f"trq{st}")
                    for p in range(4):
                        nc.tensor.transpose(trq[:, 128 * p:128 * (p + 1)],
                                            qr[:, 128 * p:128 * (p + 1)], identB[:])
                    nc.scalar.copy(
                        qTd[:].rearrange("d (p q) -> d p q", p=4)
                        [:, :, 128 * stL:128 * (stL + 1)],
                        trq[:].rearrange("d (p s) -> d p s", p=4))
                    trk = pp.tile([128, 512], BF16, tag="stagea", bufs=2,
                                  name=f"trk{st}")
                    for p in range(4):
                        nc.tensor.transpose(trk[:, 128 * p:128 * (p + 1)],
                                            kr[:, 128 * p:128 * (p + 1)], identB[:])
                    nc.vector.tensor_copy(
                        kT_all[:].rearrange("d (p t) -> d p t", p=4)
                        [:, :, 128 * st:128 * (st + 1)],
                        trk[:].rearrange("d (p s) -> d p s", p=4))
                    yield
                    # U4: v-projection + V' copy
                    pv = pp.tile([128, 512], F32, tag="stagea", bufs=2, name=f"pv{st}")
                    for ic in range(NI):
                        nc.tensor.matmul(pv[:], xv[:, ic, :], wv_t[:, ic, :],
                                         start=(ic == 0), stop=(ic == NI - 1))
                    nc.scalar.copy(
                        Vp[st][:].rearrange("p (h c) -> p h c", h=H_LOC)[:, :, 0:64],
                        pv[:].rearrange("p (h c) -> p h c", h=H_LOC))
                    yield

            # ---- attention: QK -> exp -> PV, 2-tile software pipeline ----
            def gen_attention(qc):
                if qc == 0:   # woT is first needed by gen_proj(0), much later
                    nc.sync.dma_start(
                        woT_t[:], woT[:].rearrange("(i p) c -> p i c", p=128))
                qTd = qT[qc % 3]
                OT_cur = [sp.tile([128, 512], BF16, tag="OT", bufs=8,
                                  name=f"OT{qc}_{p}") for p in range(4)]
                OT_store[qc] = OT_cur
                for pair in range(4):
                    hA, hB = 2 * pair, 2 * pair + 1
                    tiles = ([(tc, 0, False) for tc in range(4 * qc)]
                             + [(4 * qc + i, 128 * i, True) for i in range(4)])
                    nt = len(tiles)
                    pot = pp.tile([128, 1024], F32, tag="pot", bufs=1,
                                  name=f"pot{qc}_{pair}")

                    def emit_pv(ent):
                        j, tc, off, pb = ent
                        for h, colb in ((hA, 0), (hB, 512)):
                            nc.tensor.matmul(
                                pot[0:65, colb + off:colb + 512],
                                Vp[tc][:, 65 * h:65 * (h + 1)],
                                pb[:, colb + off:colb + 512],
                                start=(j == 0), stop=(j == nt - 1))

                    # tiles are processed two at a time: the four K=64 QK
                    # matmuls are emitted back-to-back (they row-tile into
                    # halves of the PE and their LDWEIGHTS pipeline through
                    # the free weight buffers), then the four PV matmuls of
                    # the lagged tile pair likewise run as one block.
                    pend = deque()
                    for g in range(nt // 2):
                        while len(pend) > 3:
                            emit_pv(pend.popleft())
                        ents = []
                        for j in (2 * g, 2 * g + 1):
                            tc, off, diag = tiles[j]
                            sc = pp.tile([128, 1024], F32, tag="sc", bufs=2,
                                         name=f"sc{qc}_{pair}_{j}")
                            pb = sp.tile([128, 1024], BF16, tag="pb", bufs=6,
                                         name=f"pb{qc}_{pair}_{j}")
                            for h, colb in ((hA, 0), (hB, 512)):
                                nc.tensor.matmul(
                                    sc[:, colb + off:colb + 512],
                                    kT_all[(h % 2) * 64:(h % 2) * 64 + 64,
                                           2048 * pair + 128 * tc:2048 * pair + 128 * (tc + 1)],
                                    qTd[(h % 2) * 64:(h % 2) * 64 + 64,
                                        512 * pair + off:512 * pair + 512])
                            ents.append((j, tc, off, diag, sc, pb))
                        for j, tc, off, diag, sc, pb in ents:
                            if off:
                                nc.scalar.activation(
                                    pb[:].rearrange("p (b q) -> p b q", b=2)[:, :, off:512],
                                    sc[:].rearrange("p (b q) -> p b q", b=2)[:, :, off:512],
                                    EXP, scale=0.125)
                            else:
                                nc.scalar.activation(pb[:], sc[:], EXP, scale=0.125)
                            if diag:
                                nc.vector.tensor_mul(
                                    pb[:].rearrange("p (b q) -> p b q", b=2)
                                    [:, :, off:off + 128],
                                    pb[:].rearrange("p (b q) -> p b q", b=2)
                                    [:, :, off:off + 128],
                                    tri01b[:].unsqueeze(1).broadcast_to([128, 2, 128]))
                            pend.append((j, tc, off, pb))
                        yield
                    while pend:
                        emit_pv(pend.popleft())
                    # normalize: copy psum out, invert ones-row, broadcast, scale
                    potS = sp.tile([128, 1024], F32, tag="potS", bufs=2,
                                   name=f"potS{qc}_{pair}")
                    nc.vector.tensor_copy(potS[0:65, :], pot[0:65, :])
                    # denominator reciprocal: DVE reciprocal cost scales with
                    # free size, so reshape the [1,1024] row onto 8 partitions
                    # via tiny SBUF DMAs (8x cheaper than recip on the row)
                    r8 = sp.tile([8, 128], F32, tag="r8", bufs=2,
                                 name=f"r8{qc}_{pair}")
                    nc.sync.dma_start(r8[:], potS[64:65, :])
                    rec8 = sp.tile([8, 128], F32, tag="rec8", bufs=2,
                                   name=f"rec8{qc}_{pair}")
                    nc.vector.reciprocal(rec8[:], r8[:])
                    rr = sp.tile([1, 1024], F32, tag="rr", bufs=2,
                                 name=f"rr{qc}_{pair}")
                    nc.sync.dma_start(rr[:], rec8[:])
                    bc = sp.tile([64, 1024], F32, tag="bc", bufs=2,
                                 name=f"bc{qc}_{pair}")
                    nc.gpsimd.partition_broadcast(bc[:], rr[:])
                    nc.vector.tensor_mul(OT_cur[pair][0:64, :],
                                         potS[0:64, 0:512], bc[:, 0:512])
                    nc.vector.tensor_mul(OT_cur[pair][64:128, :],
                                         potS[0:64, 512:1024], bc[:, 512:1024])
                    if DEBUG_DUMPS:
                        idx = 4 * qc + pair
                        nc.sync.dma_start(dbg_potS[idx], potS[0:65, :])
                        nc.sync.dma_start(dbg_rr[idx:idx + 1, :], rr[:])
                    yield
                    if qc == QC - 1:
                        # last chunk: fold this pair's slice of the output
                        # projection in right away (accumulate in SBUF) so
                        # no monolithic projection is left for the epilogue
                        for stL in range(4):
                            for half in range(2):
                                po3 = pp.tile([128, 512], F32, tag="stagea",
                                              bufs=2,
                                              name=f"po3_{pair}_{stL}_{half}")
                                nc.tensor.matmul(
                                    po3[:],
                                    OT_cur[pair][:, 128 * stL:128 * (stL + 1)],
                                    woT_t[:, pair, 512 * half:512 * (half + 1)])
                                dst = po_sb[stL][:, 512 * half:512 * (half + 1)]
                                if pair == 0:
                                    nc.vector.tensor_copy(dst, po3[:])
                                else:
                                    nc.vector.tensor_add(dst, dst, po3[:])
                            if stL % 2 == 1:
                                yield
                        if pair == 3:
                            for stL in range(4):
                                nc.sync.dma_start(
                                    out[128 * (12 + stL):128 * (13 + stL), :],
                                    po_sb[stL][:])

            # ---- output projection --------------------------------------
            def gen_proj(qc):
                OT_cur = OT_store[qc]
                for stL in range(4):
                    st = 4 * qc + stL
                    for half in range(2):
                        po = pp.tile([128, 512], F32, tag="stagea", bufs=2,
                                     name=f"po{qc}_{stL}_{half}")
                        for p in range(4):
                            nc.tensor.matmul(
                                po[:], OT_cur[p][:, 128 * stL:128 * (stL + 1)],
                                woT_t[:, p, 512 * half:512 * (half + 1)],
                                start=(p == 0), stop=(p == 3))
                        osb = sp.tile([128, 512], F32, tag="osb", bufs=4,
                                      name=f"osb{qc}_{stL}_{half}")
                        if half == 0:
                            nc.scalar.copy(osb[:], po[:])
                        else:
                            nc.vector.tensor_copy(osb[:], po[:])
                        nc.sync.dma_start(
                            out[128 * st:128 * (st + 1),
                                512 * half:512 * (half + 1)],
                            osb[:])
                        yield

            # ---- interleaved emission -----------------------------------
            # Per q-chunk, attention units (ACT-heavy) are merged with the
            # next chunk's stage-A and previous chunk's projection (both
            # PE-heavy) in proportion, so the PE queue never drains while
            # the scalar engine works through the exps.
            _SENT = object()
            if not INTERLEAVE:
                for qc in range(QC):
                    for _ in gen_stage_a(qc):
                        pass
                    for _ in gen_attention(qc):
                        pass
                    if qc < QC - 1:
                        for _ in gen_proj(qc):
                            pass
            else:
                for _ in gen_stage_a(0):        # prologue
                    pass
                # global filler queue: [generator, units_left, deadline_qc].
                # Entries are consumed FIFO, paced against attention units;
                # whatever is due by the current chunk is drained at its end.
                work = deque()
                work.append([gen_stage_a(1), 16, 0])
                work.append([gen_stage_a(2), 16, 1])

                def pump():
                    while work:
                        if next(work[0][0], _SENT) is _SENT:
                            work.popleft()
                        else:
                            work[0][1] -= 1
                            return True
                    return False

                for qc in range(QC):
                    att = gen_attention(qc)
                    if qc >= 1:
                        work.append([gen_proj(qc - 1), 8, qc])
                    # stage_a(k) writes qT[k % 3], which attention(k - 3)
                    # still reads: it must not be pushed before attention
                    # (k - 2) starts.
                    if qc >= 1 and qc + 2 <= QC - 1:
                        work.append([gen_stage_a(qc + 2), 16, qc + 1])
                    a_n = 4 * (2 * qc + 3) + (8 if qc == QC - 1 else 0)
                    f_n = sum(e[1] for e in work)
                    a_done = f_done = 0
                    for _ in att:
                        a_done += 1
                        target = (f_n * a_done) // a_n
                        while f_done < target and pump():
                            f_done += 1
                    while work and work[0][2] <= qc:
                        if next(work[0][0], _SENT) is _SENT:
                            work.popleft()
                while work:
                    if next(work[0][0], _SENT) is _SENT:
                        work.popleft()
            if DEBUG_DUMPS:
                nc.sync.dma_start(dbg_kT[:], kT_all[:])
                nc.sync.dma_start(dbg_qT[0], qT[0][:])
                nc.sync.dma_start(dbg_qT[1], qT[1][:])
                for i, st in enumerate((0, 5, 10, 15)):
                    nc.sync.dma_start(dbg_Vp[i], Vp[st][:])

    nc.compile()
    return nc


def _get_program():
    if "prog" not in _PROGRAM_CACHE:
        _PROGRAM_CACHE["prog"] = _build_program()
    return _PROGRAM_CACHE["prog"]


def _host_inputs(x, token_positions, wq, wk, wv, wo):
    import ml_dtypes
    BF = ml_dtypes.bfloat16
    x = np.asarray(x, dtype=np.float32)
    pos = np.asarray(token_positions)
    wq = np.asarray(wq, dtype=np.float32)
    wk = np.asarray(wk, dtype=np.float32)
    wv = np.asarray(wv, dtype=np.float32)
    wo = np.asarray(wo, dtype=np.float32)

    perm64 = np.concatenate([np.arange(0, 64, 2), np.arange(1, 64, 2)])
    # row selection for q/k with per-head de-interleave
    rows_perm = (np.arange(NUM_HEADS)[:, None] * 64 + perm64[None, :]).reshape(-1)
    wq_p = wq[rows_perm]            # (1024, 1024) permuted out-dims
    wk_p = wk[rows_perm]

    inv_freq = THETA ** (-np.arange(0, HEAD_DIM, 2, dtype=np.float32) / HEAD_DIM)
    ang = pos.astype(np.float32)[:, :, None] * inv_freq[None, None, :]  # (B,S,32)
    cosP = np.cos(ang, dtype=np.float32)
    sinP = np.sin(ang, dtype=np.float32)
    cosD = np.concatenate([cosP, cosP], axis=2)                  # (B,S,64)
    sinS = np.concatenate([-sinP, sinP], axis=2)

    in_maps = []
    for c in range(N_CORES):
        b, hg = c // 2, c % 2
        hsel = slice(512 * hg, 512 * (hg + 1))
        in_maps.append({
            "xT": np.ascontiguousarray(x[b].T.astype(BF)),
            "wqk": np.ascontiguousarray(
                np.concatenate([wq_p[hsel].T, wk_p[hsel].T], axis=1).astype(BF)),
            "wv": np.ascontiguousarray(wv[hsel].T.astype(BF)),
            "woT": np.ascontiguousarray(wo[:, hsel].T.astype(BF)),
            "cosD": np.ascontiguousarray(cosD[b]),
            "sinS": np.ascontiguousarray(sinS[b]),
        })
    return in_maps


def kernel(x, token_positions, wq, wk, wv, wo, _trace=False):
    nc = _get_program()
    in_maps = _host_inputs(x, token_positions, wq, wk, wv, wo)
    res = run_bass_kernel_spmd(nc, in_maps, core_ids=list(range(N_CORES)),
                               trace=_trace)
    parts = [r["out"] for r in res.results]
    out = np.stack([parts[2 * b] + parts[2 * b + 1] for b in range(BATCH)])
    kernel._last_result = res
    return out.astype(np.float32)

